# revision 17
# baseline (speedup 1.0000x reference)
"""Trainium2 Bass kernel for nn_KernelGraphCalcLayer (GNN message passing).

Computation (per batch b):
    h = relu(node_feats @ weight + bias)            # (N, OUT_DIM)
    h = h.reshape(N, K, DK)
    out[n, k, d] = sum_m adj[k, n, m] * h[m, k, d]  # per-kernel dense aggregation

Sharding: batch dim (64) split across 8 NeuronCores, 8 batches per core.
No cross-device communication.

Per-core dataflow (DMA-bound: 16MB adj + 4MB x + 1MB W reads, 2MB bf16
out writes per core):
  - ALL bulk loads ride the two HWDGE queues (sync + scalar) as fp32:
    SWDGE cast-DMA measures only ~60GB/s write-side, so x/W cast-loads
    there starve the pipeline.  SWDGE carries just bias + most out stores.
  - adj: 2 k-pairs per queue per batch, rows packed 2-per-partition
    (rows 2p, 2p+1 are HBM-contiguous -> 2KB descriptors).  x: fp32 on
    sync.  W: one fp32 half per queue up front, single DVE cast to bf16.
  - The 2-row packing makes transposed-adj free columns map to nodes
    2p+two; aggregation psum banks hold even/odd node tiles and the
    store uses a strided row view.  Casts to bf16 happen in the
    PSUM->SBUF transpose drains (bit-exact movement through the PE).
  - Per-batch PE order: xT (fp32r), linear, then adj-transpose groups
    software-pipelined PIPE=3 ahead of the aggregation matmuls: drains
    get ~1.5us to land before their matmuls and LDWEIGHTS bursts stay
    interleaved with matmul streams (dense transpose blocks trip the
    power throttle).
  - Engine split: DVE owns xT/aT drains only (nothing on DVE waits on
    aggregation matmuls); ScalarE: relu, 2 aT drains, po casts; SWDGE:
    stores (last two batches store via the by-then-idle HWDGE queues).
  - PSUM: 8 banks = 3 adj-transpose staging + 3 shared xT-staging/linear
    (alloc order ptx,ph0,ph1 staggers reuse safely) + 2 aggregation.
"""

import numpy as np

import concourse.bass as bass
import concourse.mybir as mybir
from concourse import bacc
import concourse.tile as tile
from concourse.bass_utils import run_bass_kernel_spmd
from concourse.masks import make_identity

B, N, IN_DIM, OUT_DIM, K = 64, 256, 512, 512, 8
DK = OUT_DIM // K
N_CORES = 8
BPC = B // N_CORES  # batches per core

FP32 = mybir.dt.float32
FP32R = mybir.dt.float32r
BF16 = mybir.dt.bfloat16
CDT = mybir.dt.bfloat16  # compute dtype for matmul operands
P = 128  # SBUF partitions

_compiled = {}


def _build(cdt=CDT):
    nc = bacc.Bacc("TRN2", target_bir_lowering=False, debug=False)
    x_ap = nc.dram_tensor("node_feats", [BPC, N, IN_DIM], FP32R, kind="ExternalInput").ap()
    adj_ap = nc.dram_tensor("adj", [BPC, K, N, N], FP32R, kind="ExternalInput").ap()
    w_ap = nc.dram_tensor("weight", [IN_DIM, OUT_DIM], FP32, kind="ExternalInput").ap()
    b_ap = nc.dram_tensor("bias", [OUT_DIM], FP32, kind="ExternalInput").ap()
    out_ap = nc.dram_tensor("out", [BPC, N, OUT_DIM], BF16, kind="ExternalOutput").ap()

    NC2 = N // P       # 2 node chunks of 128
    IC4 = IN_DIM // P  # 4 input-feature chunks
    NPAIR = K // 2     # 4 k-pairs per batch
    PF = 3             # batches of prefetch issued ahead
    # process k in pair-arrival order: sync delivers kp0 then kp1,
    # scalar kp2 then kp3, roughly interleaved in time
    KORDER = [0, 1, 4, 5, 2, 3, 6, 7]
    PIPE = 3           # transpose groups issued ahead of their matmuls

    with tile.TileContext(nc) as tc:
        with (
            tc.tile_pool(name="singles", bufs=1) as singles,
            tc.tile_pool(name="p_x", bufs=4) as p_x,
            tc.tile_pool(name="p_xt", bufs=2) as p_xt,
            tc.tile_pool(name="p_h", bufs=4) as p_h,
            tc.tile_pool(name="p_adj", bufs=16) as p_adj,
            tc.tile_pool(name="p_adjt", bufs=10) as p_adjt,
            tc.tile_pool(name="p_out", bufs=4) as p_out,
            tc.tile_pool(name="ps_ta", bufs=4, space=bass.MemorySpace.PSUM) as ps_ta,
            tc.tile_pool(name="ps_h", bufs=2, space=bass.MemorySpace.PSUM) as ps_h,
            tc.tile_pool(name="ps_o", bufs=2, space=bass.MemorySpace.PSUM) as ps_o,
        ):
            # --- W quarters: first instructions on both HWDGE queues ---
            w_st = [singles.tile([P, OUT_DIM], FP32, name=f"wst{i}")
                    for i in range(IC4)]
            for i in range(IC4):
                eng = nc.sync if i % 2 == 0 else nc.scalar
                eng.dma_start(out=w_st[i][:], in_=w_ap[i * P:(i + 1) * P, :])

            # --- constants ---
            id_src = singles.tile([P, P], FP32)
            make_identity(nc, id_src[:])
            id_f = singles.tile([P, P], FP32R)    # identity for fp32r transposes
            nc.vector.tensor_copy(id_f[:], id_src[:])
            ones_row = singles.tile([1, P], cdt)
            nc.gpsimd.memset(ones_row[:], 1.0)
            bias_c = singles.tile([1, OUT_DIM], cdt)
            nc.gpsimd.dma_start(out=bias_c[:], in_=b_ap[None, :])
            # w_all[:, ic*512:+512] = W[ic*128:(ic+1)*128, :] in bf16
            w_all = singles.tile([P, IC4 * OUT_DIM], cdt)
            for i in range(IC4):
                nc.vector.tensor_copy(
                    w_all[:, i * OUT_DIM:(i + 1) * OUT_DIM], w_st[i][:])

            def w_sl(ic):
                return w_all[:, ic * OUT_DIM:(ic + 1) * OUT_DIM]

            # DRAM views
            # x: [BPC, 128, 2, 512]; partition p <- nodes p, 128+p
            x_v = x_ap.rearrange("b (c p) i -> b p c i", p=P)
            # adj: [BPC, 128, K, 512]; partition p <- rows 2p, 2p+1 of each
            # k slice (contiguous 2KB in HBM)
            adj_v = adj_ap.rearrange("b k (p two) m -> b p k (two m)", two=2)
            # out: [BPC, 2, 128, OUT]; parity-two tile row p <- node 2p+two
            out_v = out_ap.rearrange("b (p two) o -> b two p o", two=2)

            pref = {}

            def prefetch(b):
                # x first on its queue: the xT transposes open every
                # batch's PE program
                x_sb = p_x.tile([P, NC2 * IN_DIM], FP32R, tag="x", name=f"x{b}")
                nc.sync.dma_start(out=x_sb[:], in_=x_v[b])
                a_sbs = []
                for kp in range(NPAIR):
                    eng = nc.sync if kp < NPAIR // 2 else nc.scalar
                    t = p_adj.tile([P, 2 * 2 * N], FP32R, tag="adj",
                                   name=f"a{b}_{kp}")
                    eng.dma_start(out=t[:], in_=adj_v[b, :, 2 * kp:2 * kp + 2])
                    a_sbs.append(t)
                pref[b] = (a_sbs, x_sb)

            for b in range(PF):
                prefetch(b)

            for b in range(BPC):
                a_sbs, x_sb = pref.pop(b)

                po = [ps_o.tile([P, OUT_DIM], FP32, tag="pso", name=f"po{b}_{i}")
                      for i in range(2)]
                aTs = {}

                def t_group(gi):
                    k = KORDER[gi]
                    kp, kl = divmod(k, 2)
                    a_sb = a_sbs[kp]
                    aT = p_adjt.tile([P, 4 * P], cdt, tag="adjT",
                                     name=f"aT{b}_{k}")
                    pt = ps_ta.tile([P, 4 * P], FP32R, tag="pstf",
                                    name=f"pta{b}_{k}")
                    for two in range(2):
                        for j in range(2):
                            blk = (two * 2 + j) * P
                            src = kl * 2 * N + two * N + j * P
                            nc.tensor.transpose(
                                pt[:, blk:blk + P], a_sb[:, src:src + P],
                                id_f[:])
                    nc.vector.tensor_copy(aT[:], pt[:])
                    aTs[k] = aT

                def m_group(gi):
                    k = KORDER[gi]
                    aT = aTs.pop(k)
                    for two in range(2):
                        for j in range(2):
                            blk = (two * 2 + j) * P
                            nc.tensor.matmul(
                                po[two][:, k * DK:(k + 1) * DK],
                                aT[:, blk:blk + P],
                                h_sb[j][:, k * DK:(k + 1) * DK],
                                start=(j == 0), stop=(j == 1))

                # --- transpose x -> xT (fp32r blocks, bf16 drains) ---
                xt = p_xt.tile([P, NC2 * IC4 * P], cdt, tag="xT", name=f"xT{b}")
                for nch in range(NC2):
                    ptx = ps_ta.tile([P, IC4 * P], FP32R, tag="pstf",
                                     name=f"ptx{b}_{nch}")
                    for ic in range(IC4):
                        nc.tensor.transpose(
                            ptx[:, ic * P:(ic + 1) * P],
                            x_sb[:, nch * IN_DIM + ic * P:
                                 nch * IN_DIM + (ic + 1) * P],
                            id_f[:])
                    nc.vector.tensor_copy(
                        xt[:, nch * IC4 * P:(nch + 1) * IC4 * P], ptx[:])

                # --- linear + bias + relu -> h bf16 [128(n), 512(o)] x2 ---
                h_sb = []
                for nch in range(NC2):
                    ph = ps_h.tile([P, OUT_DIM], FP32, tag="psh",
                                   name=f"ph{b}_{nch}")
                    nc.tensor.matmul(ph[:], ones_row[:], bias_c[:],
                                     start=True, stop=False)
                    for ic in range(IC4):
                        nc.tensor.matmul(
                            ph[:], xt[:, (nch * IC4 + ic) * P:
                                      (nch * IC4 + ic + 1) * P], w_sl(ic),
                            start=False, stop=(ic == IC4 - 1))
                    ht = p_h.tile([P, OUT_DIM], cdt, tag="h", name=f"h{b}_{nch}")
                    nc.scalar.activation(ht[:], ph[:],
                                         mybir.ActivationFunctionType.Relu)
                    h_sb.append(ht)

                # --- aggregation: transpose groups pipelined PIPE ahead ---
                for gi in range(K + PIPE):
                    if gi < K:
                        t_group(gi)
                    if gi >= PIPE:
                        m_group(gi - PIPE)

                # --- drain accumulators (ScalarE cast bf16) + store ---
                for two in range(2):
                    ot = p_out.tile([P, OUT_DIM], cdt, tag="o", name=f"o{b}_{two}")
                    nc.scalar.copy(ot[:], po[two][:])
                    if b < BPC - 2:
                        nc.gpsimd.dma_start(out=out_v[b, two], in_=ot[:])
                    elif two == 0:
                        nc.sync.dma_start(out=out_v[b, two], in_=ot[:])
                    else:
                        nc.scalar.dma_start(out=out_v[b, two], in_=ot[:])

                # prefetch LAST: every engine's blocking DMA issues (which
                # park on tile-free semaphores) trail this batch's drains
                # and casts in program order -- no head-of-line blocking of
                # compute behind prefetch
                if b + PF < BPC:
                    prefetch(b + PF)

    nc.compile()
    return nc


def _get_nc():
    if "nc" not in _compiled:
        _compiled["nc"] = _build()
    return _compiled["nc"]


def _run(inputs, trace=False, trace_cores=None):
    nc = _get_nc()
    node_feats = np.ascontiguousarray(inputs["node_feats"], dtype=np.float32)
    adj = np.ascontiguousarray(inputs["adj"], dtype=np.float32)
    weight = np.ascontiguousarray(inputs["weight"], dtype=np.float32)
    bias = np.ascontiguousarray(inputs["bias"], dtype=np.float32)
    in_maps = []
    for c in range(N_CORES):
        sl = slice(c * BPC, (c + 1) * BPC)
        in_maps.append({
            "node_feats": node_feats[sl],
            "adj": adj[sl],
            "weight": weight,
            "bias": bias,
        })
    res = run_bass_kernel_spmd(
        nc, in_maps, core_ids=list(range(N_CORES)),
        trace=trace, trace_cores=trace_cores)
    out = np.concatenate(
        [np.asarray(res.results[c]["out"]).astype(np.float32)
         for c in range(N_CORES)], axis=0)
    return out.reshape(B, N, OUT_DIM), res


def kernel(**inputs) -> np.ndarray:
    return _run(inputs, trace=False)[0]


# revision 20
# speedup vs baseline: 1.0887x; 1.0887x over previous
"""Trainium2 Bass kernel for nn_KernelGraphCalcLayer (GNN message passing).

Computation (per batch b):
    h = relu(node_feats @ weight + bias)            # (N, OUT_DIM)
    h = h.reshape(N, K, DK)
    out[n, k, d] = sum_m adj[k, n, m] * h[m, k, d]  # per-kernel dense aggregation

Sharding: batch dim (64) split across 8 NeuronCores, 8 batches per core.
No cross-device communication.

Per-core dataflow (DMA-bound: 16MB adj + 4MB x + 1MB W reads, 2MB bf16
out writes per core):
  - ALL bulk loads ride the two HWDGE queues (sync + scalar) as fp32:
    SWDGE cast-DMA measures only ~60GB/s write-side, so x/W cast-loads
    there starve the pipeline.  SWDGE carries just bias + most out stores.
  - adj: 2 k-pairs per queue per batch, rows packed 2-per-partition
    (rows 2p, 2p+1 are HBM-contiguous -> 2KB descriptors).  x: fp32 on
    sync.  W: one fp32 half per queue up front, single DVE cast to bf16.
  - The 2-row packing makes transposed-adj free columns map to nodes
    2p+two; aggregation psum banks hold even/odd node tiles and the
    store uses a strided row view.  Casts to bf16 happen in the
    PSUM->SBUF transpose drains (bit-exact movement through the PE).
  - Per-batch PE order: xT (fp32r), linear, then adj-transpose groups
    software-pipelined PIPE=3 ahead of the aggregation matmuls: drains
    get ~1.5us to land before their matmuls and LDWEIGHTS bursts stay
    interleaved with matmul streams (dense transpose blocks trip the
    power throttle).
  - Engine split: DVE owns xT/aT drains only (nothing on DVE waits on
    aggregation matmuls); ScalarE: relu, 2 aT drains, po casts; SWDGE:
    stores (last two batches store via the by-then-idle HWDGE queues).
  - PSUM: 8 banks = 3 adj-transpose staging + 3 shared xT-staging/linear
    (alloc order ptx,ph0,ph1 staggers reuse safely) + 2 aggregation.
"""

import numpy as np

import concourse.bass as bass
import concourse.mybir as mybir
from concourse import bacc
import concourse.tile as tile
from concourse.bass_utils import run_bass_kernel_spmd
from concourse.masks import make_identity

B, N, IN_DIM, OUT_DIM, K = 64, 256, 512, 512, 8
DK = OUT_DIM // K
N_CORES = 8
BPC = B // N_CORES  # batches per core

FP32 = mybir.dt.float32
FP32R = mybir.dt.float32r
BF16 = mybir.dt.bfloat16
CDT = mybir.dt.bfloat16  # compute dtype for matmul operands
P = 128  # SBUF partitions

_compiled = {}


def _build(cdt=CDT):
    nc = bacc.Bacc("TRN2", target_bir_lowering=False, debug=False)
    x_ap = nc.dram_tensor("node_feats", [BPC, N, IN_DIM], FP32R, kind="ExternalInput").ap()
    adj_ap = nc.dram_tensor("adj", [BPC, K, N, N], FP32R, kind="ExternalInput").ap()
    w_ap = nc.dram_tensor("weight", [IN_DIM, OUT_DIM], FP32, kind="ExternalInput").ap()
    b_ap = nc.dram_tensor("bias", [OUT_DIM], FP32, kind="ExternalInput").ap()
    out_ap = nc.dram_tensor("out", [BPC, N, OUT_DIM], BF16, kind="ExternalOutput").ap()

    NC2 = N // P       # 2 node chunks of 128
    IC4 = IN_DIM // P  # 4 input-feature chunks
    NPAIR = K // 2     # 4 k-pairs per batch
    PF = 3             # batches of prefetch issued ahead
    # process k in pair-arrival order: sync delivers kp0 then kp1,
    # scalar kp2 then kp3, roughly interleaved in time
    KORDER = [0, 1, 4, 5, 2, 3, 6, 7]
    PIPE = 2           # transpose groups issued ahead of their matmuls

    with tile.TileContext(nc) as tc:
        with (
            tc.tile_pool(name="singles", bufs=1) as singles,
            tc.tile_pool(name="p_x", bufs=4) as p_x,
            tc.tile_pool(name="p_xt", bufs=2) as p_xt,
            tc.tile_pool(name="p_h", bufs=4) as p_h,
            tc.tile_pool(name="p_adj", bufs=16) as p_adj,
            tc.tile_pool(name="p_adjt", bufs=10) as p_adjt,
            tc.tile_pool(name="p_out", bufs=4) as p_out,
            tc.tile_pool(name="ps_ta", bufs=4, space=bass.MemorySpace.PSUM) as ps_ta,
            tc.tile_pool(name="ps_h", bufs=2, space=bass.MemorySpace.PSUM) as ps_h,
            tc.tile_pool(name="ps_o", bufs=2, space=bass.MemorySpace.PSUM) as ps_o,
        ):
            # --- W quarters: first instructions on both HWDGE queues ---
            w_st = [singles.tile([P, OUT_DIM], FP32, name=f"wst{i}")
                    for i in range(IC4)]
            for i in range(IC4):
                eng = nc.sync if i % 2 == 0 else nc.scalar
                eng.dma_start(out=w_st[i][:], in_=w_ap[i * P:(i + 1) * P, :])

            # --- constants ---
            id_src = singles.tile([P, P], FP32)
            make_identity(nc, id_src[:])
            id_f = singles.tile([P, P], FP32R)    # identity for fp32r transposes
            nc.vector.tensor_copy(id_f[:], id_src[:])
            ones_row = singles.tile([1, P], cdt)
            nc.gpsimd.memset(ones_row[:], 1.0)
            bias_c = singles.tile([1, OUT_DIM], cdt)
            nc.gpsimd.dma_start(out=bias_c[:], in_=b_ap[None, :])
            # w_all[:, ic*512:+512] = W[ic*128:(ic+1)*128, :] in bf16
            w_all = singles.tile([P, IC4 * OUT_DIM], cdt)
            for i in range(IC4):
                nc.vector.tensor_copy(
                    w_all[:, i * OUT_DIM:(i + 1) * OUT_DIM], w_st[i][:])

            def w_sl(ic):
                return w_all[:, ic * OUT_DIM:(ic + 1) * OUT_DIM]

            # DRAM views
            # x: [BPC, 128, 2, 512]; partition p <- nodes p, 128+p
            x_v = x_ap.rearrange("b (c p) i -> b p c i", p=P)
            # adj: [BPC, 128, K, 512]; partition p <- rows 2p, 2p+1 of each
            # k slice (contiguous 2KB in HBM)
            adj_v = adj_ap.rearrange("b k (p two) m -> b p k (two m)", two=2)
            # out: [BPC, 2, 128, OUT]; parity-two tile row p <- node 2p+two
            out_v = out_ap.rearrange("b (p two) o -> b two p o", two=2)

            pref = {}

            def prefetch(b):
                # x first on its queue: the xT transposes open every
                # batch's PE program
                x_sb = p_x.tile([P, NC2 * IN_DIM], FP32R, tag="x", name=f"x{b}")
                nc.sync.dma_start(out=x_sb[:], in_=x_v[b])
                a_sbs = []
                for kp in range(NPAIR):
                    eng = nc.sync if kp < NPAIR // 2 else nc.scalar
                    t = p_adj.tile([P, 2 * 2 * N], FP32R, tag="adj",
                                   name=f"a{b}_{kp}")
                    eng.dma_start(out=t[:], in_=adj_v[b, :, 2 * kp:2 * kp + 2])
                    a_sbs.append(t)
                pref[b] = (a_sbs, x_sb)

            for b in range(PF):
                prefetch(b)

            for b in range(BPC):
                a_sbs, x_sb = pref.pop(b)

                po = [ps_o.tile([P, OUT_DIM], FP32, tag="pso", name=f"po{b}_{i}")
                      for i in range(2)]
                aTs = {}

                def t_group(gi):
                    k = KORDER[gi]
                    kp, kl = divmod(k, 2)
                    a_sb = a_sbs[kp]
                    aT = p_adjt.tile([P, 4 * P], cdt, tag="adjT",
                                     name=f"aT{b}_{k}")
                    pt = ps_ta.tile([P, 4 * P], FP32R, tag="pstf",
                                    name=f"pta{b}_{k}")
                    for two in range(2):
                        for j in range(2):
                            blk = (two * 2 + j) * P
                            src = kl * 2 * N + two * N + j * P
                            nc.tensor.transpose(
                                pt[:, blk:blk + P], a_sb[:, src:src + P],
                                id_f[:])
                    # DVE is ~co-critical at 10 drains/batch; push 2 aT
                    # drains to ScalarE (its DMA issues trail all compute in
                    # program order, so no head-of-line risk)
                    if gi in (2, 5):
                        nc.scalar.copy(aT[:], pt[:])
                    else:
                        nc.vector.tensor_copy(aT[:], pt[:])
                    aTs[k] = aT

                def m_group(gi):
                    k = KORDER[gi]
                    aT = aTs.pop(k)
                    for two in range(2):
                        for j in range(2):
                            blk = (two * 2 + j) * P
                            nc.tensor.matmul(
                                po[two][:, k * DK:(k + 1) * DK],
                                aT[:, blk:blk + P],
                                h_sb[j][:, k * DK:(k + 1) * DK],
                                start=(j == 0), stop=(j == 1))

                # --- transpose x -> xT (fp32r blocks, bf16 drains) ---
                xt = p_xt.tile([P, NC2 * IC4 * P], cdt, tag="xT", name=f"xT{b}")
                for nch in range(NC2):
                    ptx = ps_ta.tile([P, IC4 * P], FP32R, tag="pstf",
                                     name=f"ptx{b}_{nch}")
                    for ic in range(IC4):
                        nc.tensor.transpose(
                            ptx[:, ic * P:(ic + 1) * P],
                            x_sb[:, nch * IN_DIM + ic * P:
                                 nch * IN_DIM + (ic + 1) * P],
                            id_f[:])
                    nc.vector.tensor_copy(
                        xt[:, nch * IC4 * P:(nch + 1) * IC4 * P], ptx[:])

                # --- linear + bias + relu -> h bf16 [128(n), 512(o)] x2 ---
                h_sb = []
                for nch in range(NC2):
                    ph = ps_h.tile([P, OUT_DIM], FP32, tag="psh",
                                   name=f"ph{b}_{nch}")
                    nc.tensor.matmul(ph[:], ones_row[:], bias_c[:],
                                     start=True, stop=False)
                    for ic in range(IC4):
                        nc.tensor.matmul(
                            ph[:], xt[:, (nch * IC4 + ic) * P:
                                      (nch * IC4 + ic + 1) * P], w_sl(ic),
                            start=False, stop=(ic == IC4 - 1))
                    ht = p_h.tile([P, OUT_DIM], cdt, tag="h", name=f"h{b}_{nch}")
                    nc.scalar.activation(ht[:], ph[:],
                                         mybir.ActivationFunctionType.Relu)
                    h_sb.append(ht)

                # --- aggregation: transpose groups pipelined PIPE ahead ---
                for gi in range(K + PIPE):
                    if gi < K:
                        t_group(gi)
                    if gi >= PIPE:
                        m_group(gi - PIPE)

                # --- drain accumulators (ScalarE cast bf16) + store ---
                for two in range(2):
                    ot = p_out.tile([P, OUT_DIM], cdt, tag="o", name=f"o{b}_{two}")
                    nc.scalar.copy(ot[:], po[two][:])
                    if b < BPC - 2:
                        nc.gpsimd.dma_start(out=out_v[b, two], in_=ot[:])
                    elif two == 0:
                        nc.sync.dma_start(out=out_v[b, two], in_=ot[:])
                    else:
                        nc.scalar.dma_start(out=out_v[b, two], in_=ot[:])

                # prefetch LAST: every engine's blocking DMA issues (which
                # park on tile-free semaphores) trail this batch's drains
                # and casts in program order -- no head-of-line blocking of
                # compute behind prefetch
                if b + PF < BPC:
                    prefetch(b + PF)

    nc.compile()
    return nc


def _get_nc():
    if "nc" not in _compiled:
        _compiled["nc"] = _build()
    return _compiled["nc"]


def _run(inputs, trace=False, trace_cores=None):
    nc = _get_nc()
    node_feats = np.ascontiguousarray(inputs["node_feats"], dtype=np.float32)
    adj = np.ascontiguousarray(inputs["adj"], dtype=np.float32)
    weight = np.ascontiguousarray(inputs["weight"], dtype=np.float32)
    bias = np.ascontiguousarray(inputs["bias"], dtype=np.float32)
    in_maps = []
    for c in range(N_CORES):
        sl = slice(c * BPC, (c + 1) * BPC)
        in_maps.append({
            "node_feats": node_feats[sl],
            "adj": adj[sl],
            "weight": weight,
            "bias": bias,
        })
    res = run_bass_kernel_spmd(
        nc, in_maps, core_ids=list(range(N_CORES)),
        trace=trace, trace_cores=trace_cores)
    out = np.concatenate(
        [np.asarray(res.results[c]["out"]).astype(np.float32)
         for c in range(N_CORES)], axis=0)
    return out.reshape(B, N, OUT_DIM), res


def kernel(**inputs) -> np.ndarray:
    return _run(inputs, trace=False)[0]
